# revision 21
# baseline (speedup 1.0000x reference)
"""Hard-triplet miner for Trainium2, 8-core SPMD — label-sorted layout.

Host: stable-sort rows by label; each anchor's same-label set becomes a
contiguous column window around the diagonal, so the positive side only
needs a narrow slice.  keep is exact from label window sizes.

Per core / per 128-row tile (rt), mask_mode="dve":
  PE:     fp32 Gram strip G = xs_rt . xs_all^T into PSUM (8 x 1024)
          + a POSW-col Gram slice around the diagonal block (pos).
  Scalar: label masks: nesc = |lab_j - lab_i| (u8),
          eqsc = relu(2 - 2*nesc) in {2, 0} (bf16, exact).
  DVE:    vals = G - eqsc (tensor_tensor, exact for diff-label entries);
          gmax = max(vals) (hardest negative; same-label pushed to
          G-2 <= -1); wpos = G_slice - eqsc_pos, vpos = min(wpos)
          (hardest positive = min same-label G, diff-label >= -1 can
          never win); one uint16 max_index over the marker array.
  Pool:   markers: 1 where vals == gmax, 2 where wpos == vpos.

mask_mode="pe": the -2*same shift is applied inside the PE by one-hot
label matmuls accumulating onto the Gram PSUM; DVE reduces straight
from PSUM and the Scalar engine makes the SBUF copies for the markers.

Host decodes sorted positions through the permutation.
"""

import numpy as np

import concourse.bacc as bacc
import concourse.bass as bass
import concourse.mybir as mybir
import concourse.tile as tile
from concourse import masks
from concourse.bass_utils import run_bass_kernel_spmd

F32 = mybir.dt.float32
BF16 = mybir.dt.bfloat16
U16 = mybir.dt.uint16
U8 = mybir.dt.uint8

N = 8192
D = 128
NCORES = 8
STRIP = N // NCORES        # 1024 anchor rows per core
RT = STRIP // 128          # 8 row tiles per core
CTW = 1024                 # column-tile width (PSUM tile)
CT = N // CTW              # 8 column tiles
POSW = 256                 # positive-window slice width
MKW = N + POSW             # marker array width
FLT_LOW = -3.4028234663852886e38
FLT_HIGH = 3.4028234663852886e38


def build_program(k_repeat: int = 1, use_for_i: bool = False,
                  mask_mode: str = "dve", oh_dt=BF16, dbg: str = ""):
    pe_mask = mask_mode == "pe"
    nc = bacc.Bacc("TRN2", target_bir_lowering=False, debug=False,
                   num_devices=NCORES)

    x_full = nc.dram_tensor("x_full", [N, D], F32, kind="ExternalInput")
    x_strip = nc.dram_tensor("x_strip", [STRIP, D], F32, kind="ExternalInput")
    x_pos = nc.dram_tensor("x_pos", [RT * POSW, D], F32, kind="ExternalInput")
    if pe_mask:
        oh_full = [nc.dram_tensor(f"oh_full{h}", [128, N], F32,
                                  kind="ExternalInput") for h in range(2)]
        oh_strip = [nc.dram_tensor(f"oh_strip{h}", [128, STRIP], F32,
                                   kind="ExternalInput") for h in range(2)]
        oh_pos = [nc.dram_tensor(f"oh_pos{h}", [128, RT * POSW], F32,
                                 kind="ExternalInput") for h in range(2)]
    else:
        lab_full = nc.dram_tensor("lab_full", [1, N], F32,
                                  kind="ExternalInput")
        lab_pos = nc.dram_tensor("lab_pos", [1, RT * POSW], F32,
                                 kind="ExternalInput")
        lab_strip = nc.dram_tensor("lab_strip", [128, RT], F32,
                                   kind="ExternalInput")
    stage_out = nc.dram_tensor("stage_out", [128, RT * 8], U16,
                               kind="ExternalOutput")

    with tile.TileContext(nc) as tc:
        with (
            tc.tile_pool(name="persist", bufs=1) as persist,
            tc.tile_pool(name="rowp", bufs=3) as rowp,
            tc.tile_pool(name="nescp", bufs=1) as nescp,
            tc.tile_pool(name="eqp", bufs=1) as eqp,
            tc.tile_pool(name="valsp", bufs=2) as valsp,
            tc.tile_pool(name="marksp", bufs=2) as marksp,
            tc.tile_pool(name="wposp", bufs=2) as wposp,
            tc.tile_pool(name="smalls", bufs=4) as smalls,
            tc.tile_pool(name="psum_g", bufs=3,
                         space=bass.MemorySpace.PSUM) as psum_g,
            tc.tile_pool(name="psum_pos", bufs=2,
                         space=bass.MemorySpace.PSUM) as psum_pos,
        ):
            ident = persist.tile([128, 128], F32)
            masks.make_identity(nc, ident[:])

            xT = persist.tile([128, N], F32, tag="xT")
            xsT = persist.tile([128, STRIP], F32, tag="xsT")
            xTpos = persist.tile([128, RT * POSW], F32, tag="xTpos")

            bias0 = persist.tile([128, 1], F32, tag="bias0")
            nc.gpsimd.memset(bias0[:], 0.0)
            bias2 = persist.tile([128, 1], F32, tag="bias2")
            nc.gpsimd.memset(bias2[:], 2.0)

            # match-marker values for max_index: slot0 -> 1 (neg),
            # slot1 -> 2 (pos), rest padding that never occurs
            inmax = persist.tile([128, 8], U16, tag="inmax")
            nc.gpsimd.memset(inmax[:], 9)
            nc.gpsimd.memset(inmax[:, 0:1], 1)
            nc.gpsimd.memset(inmax[:, 1:2], 2)

            stage = persist.tile([128, RT * 8], U16, tag="stage")

            def norm_transpose(dst, src_dram, tiles):
                for t in range(tiles):
                    row = rowp.tile([128, D], F32, tag="row")
                    nc.sync.dma_start(row[:], src_dram[t * 128:(t + 1) * 128, :])
                    sq = rowp.tile([128, D], F32, tag="sq")
                    ssq = smalls.tile([128, 1], F32, tag="ssq")
                    nc.scalar.activation(sq[:], row[:],
                                         mybir.ActivationFunctionType.Square,
                                         bias=bias0[:], accum_out=ssq[:])
                    nrm = smalls.tile([128, 1], F32, tag="nrm")
                    nc.scalar.activation(nrm[:], ssq[:],
                                         mybir.ActivationFunctionType.Sqrt,
                                         bias=bias0[:])
                    rin = smalls.tile([128, 1], F32, tag="rin")
                    nc.vector.reciprocal(rin[:], nrm[:])
                    xn = rowp.tile([128, D], F32, tag="xn")
                    nc.vector.tensor_scalar_mul(xn[:], row[:], rin[:])
                    pt = psum_pos.tile([128, POSW], F32, tag="pp")
                    nc.tensor.transpose(pt[:, 0:128], xn[:], ident[:])
                    nc.scalar.activation(dst[:, t * 128:(t + 1) * 128],
                                         pt[:, 0:128],
                                         mybir.ActivationFunctionType.Copy)

            norm_transpose(xT, x_full, N // 128)
            norm_transpose(xsT, x_strip, STRIP // 128)
            norm_transpose(xTpos, x_pos, RT * POSW // 128)

            if pe_mask:
                # one-hot label tensors (host-built): stationary halves are
                # scaled by -2, so accumulating onto the Gram PSUM applies
                # the -2*[same-label] shift exactly (0.0 added to diff).
                ohT = [persist.tile([128, N], oh_dt, tag=f"ohT{h}")
                       for h in range(2)]
                ohsT = [persist.tile([128, STRIP], oh_dt, tag=f"ohsT{h}")
                        for h in range(2)]
                ohTp = [persist.tile([128, RT * POSW], oh_dt, tag=f"ohTp{h}")
                        for h in range(2)]

                def load_cast(dst, dram, width):
                    for c in range(width // 2048):
                        f = rowp.tile([128, 2048], F32, tag="ohstage")
                        nc.sync.dma_start(f[:], dram[:, c * 2048:(c + 1) * 2048])
                        nc.vector.tensor_copy(dst[:, c * 2048:(c + 1) * 2048],
                                              f[:])

                for h in range(2):
                    load_cast(ohT[h], oh_full[h], N)
                    load_cast(ohsT[h], oh_strip[h], STRIP)
                    load_cast(ohTp[h], oh_pos[h], RT * POSW)
            else:
                labrep = persist.tile([128, N], U8, tag="labrep")
                labrep_pos = persist.tile([128, RT * POSW], U8,
                                          tag="labrep_pos")
                labsT = persist.tile([128, RT], F32, tag="labsT")
                ones1 = persist.tile([1, 128], F32, tag="ones1")
                nc.gpsimd.memset(ones1[:], 1.0)
                nc.sync.dma_start(labsT[:], lab_strip[:])

                def lab_broadcast(dst, src_dram, n_cols):
                    for c in range(n_cols // 256):
                        l1 = rowp.tile([1, 256], F32, tag="labchunk")
                        nc.sync.dma_start(l1[:],
                                          src_dram[:, c * 256:(c + 1) * 256])
                        pl = psum_pos.tile([128, POSW], F32, tag="pp")
                        nc.tensor.matmul(pl[:, 0:256], ones1[:], l1[:])
                        nc.scalar.activation(
                            dst[:, c * 256:(c + 1) * 256], pl[:, 0:256],
                            mybir.ActivationFunctionType.Copy)

                lab_broadcast(labrep, lab_full, N)
                lab_broadcast(labrep_pos, lab_pos, RT * POSW)

                labsTm1 = persist.tile([128, RT], F32, tag="labsTm1")
                nc.vector.tensor_scalar_mul(labsTm1[:], labsT[:], -1.0)

            def main_body():
                pending = []  # (rt, marks) awaiting max_index, 1-deep

                def flush_mi():
                    while pending:
                        prt, pmarks = pending.pop(0)
                        if "nomi" in dbg:
                            nc.vector.memset(
                                stage[:, prt * 8:(prt + 1) * 8], 0)
                        else:
                            nc.vector.max_index(
                                stage[:, prt * 8:(prt + 1) * 8],
                                inmax[:], pmarks[:])

                for rt in range(RT):
                    vals = valsp.tile([128, N], F32, tag="vals")
                    marks = marksp.tile([128, MKW], U16, tag="marks")
                    wpos = wposp.tile([128, POSW], F32, tag="wpos")
                    acc = smalls.tile([128, CT], F32, tag="acc")
                    gmax = smalls.tile([128, 1], F32, tag="gmax")
                    vpos = smalls.tile([128, 1], F32, tag="vpos")
                    stat = xsT[:, rt * 128:(rt + 1) * 128]

                    if not pe_mask:
                        # label masks (scalar engine)
                        nesc = nescp.tile([128, N], U8, tag="nesc")
                        nc.scalar.activation(nesc[:], labrep[:],
                                             mybir.ActivationFunctionType.Abs,
                                             scale=1.0,
                                             bias=labsTm1[:, rt:rt + 1])
                        eqsc = eqp.tile([128, N], BF16, tag="eqsc")
                        nc.scalar.activation(eqsc[:], nesc[:],
                                             mybir.ActivationFunctionType.Relu,
                                             scale=-2.0, bias=bias2[:])
                        nescp_t = nescp.tile([128, POSW], U8, tag="nesc_pos")
                        nc.scalar.activation(
                            nescp_t[:],
                            labrep_pos[:, rt * POSW:(rt + 1) * POSW],
                            mybir.ActivationFunctionType.Abs,
                            scale=1.0, bias=labsTm1[:, rt:rt + 1])
                        eqscp_t = eqp.tile([128, POSW], BF16, tag="eqsc_pos")
                        nc.scalar.activation(eqscp_t[:], nescp_t[:],
                                             mybir.ActivationFunctionType.Relu,
                                             scale=-2.0, bias=bias2[:])

                    for c in range(CT):
                        ps = psum_g.tile([128, CTW], F32, tag="ps")
                        for h in range(CTW // 512):
                            lo = c * CTW + h * 512
                            if pe_mask:
                                nc.tensor.matmul(ps[:, h * 512:(h + 1) * 512],
                                                 stat, xT[:, lo:lo + 512],
                                                 start=True, stop=False)
                                for g in range(2):
                                    nc.tensor.matmul(
                                        ps[:, h * 512:(h + 1) * 512],
                                        ohsT[g][:, rt * 128:(rt + 1) * 128],
                                        ohT[g][:, lo:lo + 512],
                                        start=False, stop=(g == 1))
                            else:
                                nc.tensor.matmul(ps[:, h * 512:(h + 1) * 512],
                                                 stat, xT[:, lo:lo + 512])
                        if pe_mask:
                            # vals = G' (already shifted); per-ct max from PSUM
                            nc.vector.tensor_reduce(
                                acc[:, c:c + 1], ps[:], mybir.AxisListType.X,
                                mybir.AluOpType.max)
                            nc.scalar.activation(
                                vals[:, c * CTW:(c + 1) * CTW], ps[:],
                                mybir.ActivationFunctionType.Copy)
                        else:
                            # vals = G - eqsc
                            nc.vector.tensor_tensor(
                                vals[:, c * CTW:(c + 1) * CTW], ps[:],
                                eqsc[:, c * CTW:(c + 1) * CTW],
                                mybir.AluOpType.subtract)
                    if pe_mask:
                        nc.vector.tensor_reduce(gmax[:, 0:1], acc[:],
                                                mybir.AxisListType.X,
                                                mybir.AluOpType.max)
                    else:
                        nc.vector.tensor_reduce(gmax[:, 0:1], vals[:],
                                                mybir.AxisListType.X,
                                                mybir.AluOpType.max)

                    pp = psum_pos.tile([128, POSW], F32, tag="pp")
                    if pe_mask:
                        nc.tensor.matmul(pp[:], stat,
                                         xTpos[:, rt * POSW:(rt + 1) * POSW],
                                         start=True, stop=False)
                        for g in range(2):
                            nc.tensor.matmul(
                                pp[:],
                                ohsT[g][:, rt * 128:(rt + 1) * 128],
                                ohTp[g][:, rt * POSW:(rt + 1) * POSW],
                                start=False, stop=(g == 1))
                    else:
                        nc.tensor.matmul(pp[:], stat,
                                         xTpos[:, rt * POSW:(rt + 1) * POSW])
                    if pe_mask:
                        nc.vector.tensor_reduce(vpos[:, 0:1], pp[:],
                                                mybir.AxisListType.X,
                                                mybir.AluOpType.min)
                        nc.scalar.activation(wpos[:], pp[:],
                                             mybir.ActivationFunctionType.Copy)
                    else:
                        # wpos = G_slice - eqsc_pos; min = minG_window - 2
                        nc.vector.tensor_tensor(wpos[:], pp[:], eqscp_t[:],
                                                mybir.AluOpType.subtract)
                        nc.vector.tensor_reduce(vpos[:, 0:1], wpos[:],
                                                mybir.AxisListType.X,
                                                mybir.AluOpType.min)
                    # markers: 1 at hardest-neg matches, 2 at hardest-pos
                    if "nogp" in dbg:
                        nc.vector.memset(marks[:, 0:N], 1)
                        nc.vector.memset(marks[:, N:MKW], 2)
                    else:
                        nc.gpsimd.tensor_scalar(
                            marks[:, 0:N], vals[:], gmax[:, 0:1], None,
                            op0=mybir.AluOpType.is_equal)
                        nc.gpsimd.tensor_scalar(
                            marks[:, N:MKW], wpos[:], vpos[:, 0:1], 2.0,
                            op0=mybir.AluOpType.is_equal,
                            op1=mybir.AluOpType.mult)
                    flush_mi()
                    pending.append((rt, marks))
                flush_mi()

            if use_for_i:
                with tc.For_i(0, k_repeat, 1):
                    main_body()
            else:
                for _ in range(k_repeat):
                    main_body()

            nc.sync.dma_start(stage_out[:], stage[:])

    nc.compile()
    return nc


def _host_prep(x, labels, pe_mask=False):
    """Sort rows by label; window metadata + per-core input maps."""
    lab = np.asarray(labels).astype(np.int64)
    x = np.ascontiguousarray(np.asarray(x, dtype=np.float32))
    perm = np.argsort(lab, kind="stable")
    labs = lab[perm]
    xs = np.ascontiguousarray(x[perm])
    ws = np.searchsorted(labs, labs, "left").astype(np.int64)
    we = np.searchsorted(labs, labs, "right").astype(np.int64)
    labs_f = labs.astype(np.float32)

    if pe_mask:
        ohm = np.zeros((256, N), np.float32)
        ohm[labs, np.arange(N)] = 1.0

    in_maps = []
    los = np.zeros((NCORES, RT), dtype=np.int64)
    for m in range(NCORES):
        xpos = np.zeros((RT * POSW, D), np.float32)
        labp = np.zeros(RT * POSW, np.float32)
        labst = np.zeros((128, RT), np.float32)
        pos_cols = np.zeros(RT * POSW, np.int64)
        for rt in range(RT):
            base = m * STRIP + rt * 128
            rows = np.arange(base, base + 128)
            lo = min(max(base - (POSW - 128) // 2, 0), N - POSW)
            los[m, rt] = lo
            assert ws[rows].min() >= lo and we[rows].max() <= lo + POSW, (
                "positive window slice overflow; enlarge POSW"
            )
            xpos[rt * POSW:(rt + 1) * POSW] = xs[lo:lo + POSW]
            labp[rt * POSW:(rt + 1) * POSW] = labs_f[lo:lo + POSW]
            labst[:, rt] = labs_f[rows]
            pos_cols[rt * POSW:(rt + 1) * POSW] = np.arange(lo, lo + POSW)
        im = {
            "x_full": xs,
            "x_strip": np.ascontiguousarray(xs[m * STRIP:(m + 1) * STRIP]),
            "x_pos": xpos,
        }
        if pe_mask:
            strip_cols = np.arange(m * STRIP, (m + 1) * STRIP)
            ohs = np.ascontiguousarray(ohm[:, strip_cols]) * -2.0
            ohp = np.ascontiguousarray(ohm[:, pos_cols])
            im.update({
                "oh_full0": ohm[:128], "oh_full1": ohm[128:],
                "oh_strip0": ohs[:128], "oh_strip1": ohs[128:],
                "oh_pos0": ohp[:128], "oh_pos1": ohp[128:],
            })
        else:
            im.update({
                "lab_full": labs_f.reshape(1, N),
                "lab_pos": labp.reshape(1, RT * POSW),
                "lab_strip": labst,
            })
        in_maps.append(im)
    keep_sorted = ((we - ws) >= 2) & ((we - ws) <= N - 1)
    return in_maps, perm, los, keep_sorted


_CACHED_NC = None
_MASK_MODE = None


def kernel(l_embeds: np.ndarray, l_labels: np.ndarray):
    import os
    global _CACHED_NC, _MASK_MODE
    if _CACHED_NC is None:
        _MASK_MODE = os.environ.get("KERNEL_MASK_MODE", "dve")
        _CACHED_NC = build_program(mask_mode=_MASK_MODE)
    nc = _CACHED_NC

    lab_i = np.asarray(l_labels)
    in_maps, perm, los, keep_sorted = _host_prep(
        l_embeds, lab_i, pe_mask=_MASK_MODE == "pe")

    res = run_bass_kernel_spmd(nc, in_maps, list(range(NCORES))).results

    neg_sorted = np.empty(N, np.int64)
    pos_sorted = np.empty(N, np.int64)
    for m in range(NCORES):
        st = res[m]["stage_out"].astype(np.int64).reshape(128, RT, 8)
        for rt in range(RT):
            base = m * STRIP + rt * 128
            neg_sorted[base:base + 128] = st[:, rt, 0]
            pos_sorted[base:base + 128] = st[:, rt, 1] - N + los[m, rt]

    # sorted-space -> original-space indices
    idt = np.int64 if lab_i.dtype == np.int64 else np.int32
    anchor = np.arange(N, dtype=idt)
    neg = np.empty(N, np.int64)
    pos = np.empty(N, np.int64)
    keep = np.empty(N, bool)
    srows = np.arange(N)
    neg[perm[srows]] = perm[neg_sorted]
    pos[perm[srows]] = perm[pos_sorted]
    keep[perm[srows]] = keep_sorted
    return (anchor, pos.astype(idt), neg.astype(idt), keep)


# revision 23
# speedup vs baseline: 1.9330x; 1.9330x over previous
"""Hard-triplet miner for Trainium2, 8-core SPMD — label-sorted layout.

Host: stable-sort rows by label; each anchor's same-label set becomes a
contiguous column window around the diagonal, so the positive side only
needs a narrow slice.  keep is exact from label window sizes.

Per core / per 128-row tile (rt), mask_mode="dve":
  PE:     fp32 Gram strip G = xs_rt . xs_all^T into PSUM (8 x 1024)
          + a POSW-col Gram slice around the diagonal block (pos).
  Scalar: label masks: nesc = |lab_j - lab_i| (u8),
          eqsc = relu(2 - 2*nesc) in {2, 0} (bf16, exact).
  DVE:    vals = G - eqsc (tensor_tensor, exact for diff-label entries);
          gmax = max(vals) (hardest negative; same-label pushed to
          G-2 <= -1); wpos = G_slice - eqsc_pos, vpos = min(wpos)
          (hardest positive = min same-label G, diff-label >= -1 can
          never win); one uint16 max_index over the marker array.
  Pool:   markers: 1 where vals == gmax, 2 where wpos == vpos.

mask_mode="pe": the -2*same shift is applied inside the PE by one-hot
label matmuls accumulating onto the Gram PSUM; DVE reduces straight
from PSUM and the Scalar engine makes the SBUF copies for the markers.

Host decodes sorted positions through the permutation.
"""

import numpy as np

import concourse.bacc as bacc
import concourse.bass as bass
import concourse.mybir as mybir
import concourse.tile as tile
from concourse import masks
from concourse.bass_utils import run_bass_kernel_spmd

F32 = mybir.dt.float32
BF16 = mybir.dt.bfloat16
U16 = mybir.dt.uint16
U8 = mybir.dt.uint8

N = 8192
D = 128
NCORES = 8
STRIP = N // NCORES        # 1024 anchor rows per core
RT = STRIP // 128          # 8 row tiles per core
CTW = 512                  # column-tile width (PSUM tile)
CT = N // CTW              # 8 column tiles
POSW = 256                 # positive-window slice width
MKW = N + POSW             # marker array width
FLT_LOW = -3.4028234663852886e38
FLT_HIGH = 3.4028234663852886e38


def build_program(k_repeat: int = 1, use_for_i: bool = False,
                  mask_mode: str = "dve", oh_dt=BF16, dbg: str = ""):
    pe_mask = mask_mode == "pe"
    nc = bacc.Bacc("TRN2", target_bir_lowering=False, debug=False,
                   num_devices=NCORES)

    x_full = nc.dram_tensor("x_full", [N, D], F32, kind="ExternalInput")
    x_strip = nc.dram_tensor("x_strip", [STRIP, D], F32, kind="ExternalInput")
    x_pos = nc.dram_tensor("x_pos", [RT * POSW, D], F32, kind="ExternalInput")
    if pe_mask:
        oh_full = [nc.dram_tensor(f"oh_full{h}", [128, N], F32,
                                  kind="ExternalInput") for h in range(2)]
        oh_strip = [nc.dram_tensor(f"oh_strip{h}", [128, STRIP], F32,
                                   kind="ExternalInput") for h in range(2)]
        oh_pos = [nc.dram_tensor(f"oh_pos{h}", [128, RT * POSW], F32,
                                 kind="ExternalInput") for h in range(2)]
    else:
        lab_full = nc.dram_tensor("lab_full", [1, N], F32,
                                  kind="ExternalInput")
        lab_pos = nc.dram_tensor("lab_pos", [1, RT * POSW], F32,
                                 kind="ExternalInput")
        lab_strip = nc.dram_tensor("lab_strip", [128, RT], F32,
                                   kind="ExternalInput")
    stage_out = nc.dram_tensor("stage_out", [128, RT * 8], U16,
                               kind="ExternalOutput")

    with tile.TileContext(nc) as tc:
        with (
            tc.tile_pool(name="persist", bufs=1) as persist,
            tc.tile_pool(name="rowp", bufs=3) as rowp,
            tc.tile_pool(name="nescp", bufs=1) as nescp,
            tc.tile_pool(name="eqp", bufs=2) as eqp,
            tc.tile_pool(name="valsp", bufs=1) as valsp,
            tc.tile_pool(name="wposp", bufs=2) as wposp,
            tc.tile_pool(name="smalls", bufs=4) as smalls,
            tc.tile_pool(name="psum_g", bufs=6,
                         space=bass.MemorySpace.PSUM) as psum_g,
            tc.tile_pool(name="psum_pos", bufs=2,
                         space=bass.MemorySpace.PSUM) as psum_pos,
        ):
            ident = persist.tile([128, 128], F32)
            masks.make_identity(nc, ident[:])

            xT = persist.tile([128, N], F32, tag="xT")
            xsT = persist.tile([128, STRIP], F32, tag="xsT")
            xTpos = persist.tile([128, RT * POSW], F32, tag="xTpos")

            bias0 = persist.tile([128, 1], F32, tag="bias0")
            nc.gpsimd.memset(bias0[:], 0.0)
            bias2 = persist.tile([128, 1], F32, tag="bias2")
            nc.gpsimd.memset(bias2[:], 2.0)

            stage = persist.tile([128, RT * 8], U16, tag="stage")
            inmax = persist.tile([128, 8], F32, tag="inmax")
            nc.gpsimd.memset(inmax[:], FLT_HIGH)

            def norm_transpose(dst, src_dram, tiles):
                for t in range(tiles):
                    row = rowp.tile([128, D], F32, tag="row")
                    nc.sync.dma_start(row[:], src_dram[t * 128:(t + 1) * 128, :])
                    sq = rowp.tile([128, D], F32, tag="sq")
                    ssq = smalls.tile([128, 1], F32, tag="ssq")
                    nc.scalar.activation(sq[:], row[:],
                                         mybir.ActivationFunctionType.Square,
                                         bias=bias0[:], accum_out=ssq[:])
                    nrm = smalls.tile([128, 1], F32, tag="nrm")
                    nc.scalar.activation(nrm[:], ssq[:],
                                         mybir.ActivationFunctionType.Sqrt,
                                         bias=bias0[:])
                    rin = smalls.tile([128, 1], F32, tag="rin")
                    nc.vector.reciprocal(rin[:], nrm[:])
                    xn = rowp.tile([128, D], F32, tag="xn")
                    nc.vector.tensor_scalar_mul(xn[:], row[:], rin[:])
                    pt = psum_pos.tile([128, POSW], F32, tag="pp")
                    nc.tensor.transpose(pt[:, 0:128], xn[:], ident[:])
                    nc.scalar.activation(dst[:, t * 128:(t + 1) * 128],
                                         pt[:, 0:128],
                                         mybir.ActivationFunctionType.Copy)

            norm_transpose(xT, x_full, N // 128)
            norm_transpose(xsT, x_strip, STRIP // 128)
            norm_transpose(xTpos, x_pos, RT * POSW // 128)

            if pe_mask:
                # one-hot label tensors (host-built): stationary halves are
                # scaled by -2, so accumulating onto the Gram PSUM applies
                # the -2*[same-label] shift exactly (0.0 added to diff).
                ohT = [persist.tile([128, N], oh_dt, tag=f"ohT{h}")
                       for h in range(2)]
                ohsT = [persist.tile([128, STRIP], oh_dt, tag=f"ohsT{h}")
                        for h in range(2)]
                ohTp = [persist.tile([128, RT * POSW], oh_dt, tag=f"ohTp{h}")
                        for h in range(2)]

                def load_cast(dst, dram, width):
                    for c in range(width // 2048):
                        f = rowp.tile([128, 2048], F32, tag="ohstage")
                        nc.sync.dma_start(f[:], dram[:, c * 2048:(c + 1) * 2048])
                        nc.vector.tensor_copy(dst[:, c * 2048:(c + 1) * 2048],
                                              f[:])

                for h in range(2):
                    load_cast(ohT[h], oh_full[h], N)
                    load_cast(ohsT[h], oh_strip[h], STRIP)
                    load_cast(ohTp[h], oh_pos[h], RT * POSW)
            else:
                labrep = persist.tile([128, N], U8, tag="labrep")
                labrep_pos = persist.tile([128, RT * POSW], U8,
                                          tag="labrep_pos")
                labsT = persist.tile([128, RT], F32, tag="labsT")
                ones1 = persist.tile([1, 128], F32, tag="ones1")
                nc.gpsimd.memset(ones1[:], 1.0)
                nc.sync.dma_start(labsT[:], lab_strip[:])

                def lab_broadcast(dst, src_dram, n_cols):
                    for c in range(n_cols // 256):
                        l1 = rowp.tile([1, 256], F32, tag="labchunk")
                        nc.sync.dma_start(l1[:],
                                          src_dram[:, c * 256:(c + 1) * 256])
                        pl = psum_pos.tile([128, POSW], F32, tag="pp")
                        nc.tensor.matmul(pl[:, 0:256], ones1[:], l1[:])
                        nc.scalar.activation(
                            dst[:, c * 256:(c + 1) * 256], pl[:, 0:256],
                            mybir.ActivationFunctionType.Copy)

                lab_broadcast(labrep, lab_full, N)
                lab_broadcast(labrep_pos, lab_pos, RT * POSW)

                labsTm1 = persist.tile([128, RT], F32, tag="labsTm1")
                nc.vector.tensor_scalar_mul(labsTm1[:], labsT[:], -1.0)

            def main_body():
                for rt in range(RT):
                    vals = valsp.tile([128, N], F32, tag="vals")
                    wpos = wposp.tile([128, POSW], F32, tag="wpos")
                    acc = smalls.tile([128, CT], F32, tag="acc")
                    gmax = smalls.tile([128, 1], F32, tag="gmax")
                    vpos = smalls.tile([128, 1], F32, tag="vpos")
                    stat = xsT[:, rt * 128:(rt + 1) * 128]

                    if not pe_mask:
                        # label masks (scalar engine)
                        nesc = nescp.tile([128, N], U8, tag="nesc")
                        nc.scalar.activation(nesc[:], labrep[:],
                                             mybir.ActivationFunctionType.Abs,
                                             scale=1.0,
                                             bias=labsTm1[:, rt:rt + 1])
                        eqsc = eqp.tile([128, N], BF16, tag="eqsc")
                        nc.scalar.activation(eqsc[:], nesc[:],
                                             mybir.ActivationFunctionType.Relu,
                                             scale=-2.0, bias=bias2[:])
                        nescp_t = nescp.tile([128, POSW], U8, tag="nesc_pos")
                        nc.scalar.activation(
                            nescp_t[:],
                            labrep_pos[:, rt * POSW:(rt + 1) * POSW],
                            mybir.ActivationFunctionType.Abs,
                            scale=1.0, bias=labsTm1[:, rt:rt + 1])
                        eqscp_t = eqp.tile([128, POSW], BF16, tag="eqsc_pos")
                        nc.scalar.activation(eqscp_t[:], nescp_t[:],
                                             mybir.ActivationFunctionType.Relu,
                                             scale=-2.0, bias=bias2[:])

                    for c in range(CT):
                        ps = psum_g.tile([128, CTW], F32, tag="ps")
                        for h in range(CTW // 512):
                            lo = c * CTW + h * 512
                            if pe_mask:
                                nc.tensor.matmul(ps[:, h * 512:(h + 1) * 512],
                                                 stat, xT[:, lo:lo + 512],
                                                 start=True, stop=False)
                                for g in range(2):
                                    nc.tensor.matmul(
                                        ps[:, h * 512:(h + 1) * 512],
                                        ohsT[g][:, rt * 128:(rt + 1) * 128],
                                        ohT[g][:, lo:lo + 512],
                                        start=False, stop=(g == 1))
                            else:
                                nc.tensor.matmul(ps[:, h * 512:(h + 1) * 512],
                                                 stat, xT[:, lo:lo + 512])
                        if pe_mask:
                            # vals = G' (already shifted); per-ct max from PSUM
                            nc.vector.tensor_reduce(
                                acc[:, c:c + 1], ps[:], mybir.AxisListType.X,
                                mybir.AluOpType.max)
                            nc.scalar.activation(
                                vals[:, c * CTW:(c + 1) * CTW], ps[:],
                                mybir.ActivationFunctionType.Copy)
                        else:
                            # vals = G - eqsc
                            nc.vector.tensor_tensor(
                                vals[:, c * CTW:(c + 1) * CTW], ps[:],
                                eqsc[:, c * CTW:(c + 1) * CTW],
                                mybir.AluOpType.subtract)
                    if pe_mask:
                        nc.vector.tensor_reduce(gmax[:, 0:1], acc[:],
                                                mybir.AxisListType.X,
                                                mybir.AluOpType.max)
                    else:
                        nc.vector.tensor_reduce(gmax[:, 0:1], vals[:],
                                                mybir.AxisListType.X,
                                                mybir.AluOpType.max)

                    pp = psum_pos.tile([128, POSW], F32, tag="pp")
                    if pe_mask:
                        nc.tensor.matmul(pp[:], stat,
                                         xTpos[:, rt * POSW:(rt + 1) * POSW],
                                         start=True, stop=False)
                        for g in range(2):
                            nc.tensor.matmul(
                                pp[:],
                                ohsT[g][:, rt * 128:(rt + 1) * 128],
                                ohTp[g][:, rt * POSW:(rt + 1) * POSW],
                                start=False, stop=(g == 1))
                    else:
                        nc.tensor.matmul(pp[:], stat,
                                         xTpos[:, rt * POSW:(rt + 1) * POSW])
                    if pe_mask:
                        nc.vector.tensor_reduce(vpos[:, 0:1], pp[:],
                                                mybir.AxisListType.X,
                                                mybir.AluOpType.min)
                        nc.scalar.activation(wpos[:], pp[:],
                                             mybir.ActivationFunctionType.Copy)
                    else:
                        # wpos = G_slice - eqsc_pos; min = minG_window - 2
                        nc.vector.tensor_tensor(wpos[:], pp[:], eqscp_t[:],
                                                mybir.AluOpType.subtract)
                        nc.vector.tensor_reduce(vpos[:, 0:1], wpos[:],
                                                mybir.AxisListType.X,
                                                mybir.AluOpType.min)
                    # value-matched index extraction straight off vals:
                    # slot0 = gmax (hardest neg); slot1 = vpos — the
                    # hardest-pos value G_jmin - 2 appears bit-identically
                    # at global column jmin inside vals, so it matches there
                    nc.vector.tensor_copy(inmax[:, 0:1], gmax[:, 0:1])
                    nc.vector.tensor_copy(inmax[:, 1:2], vpos[:, 0:1])
                    if "nomi" in dbg:
                        nc.vector.memset(stage[:, rt * 8:(rt + 1) * 8], 0)
                    else:
                        nc.vector.max_index(stage[:, rt * 8:(rt + 1) * 8],
                                            inmax[:], vals[:])

            if use_for_i:
                with tc.For_i(0, k_repeat, 1):
                    main_body()
            else:
                for _ in range(k_repeat):
                    main_body()

            nc.sync.dma_start(stage_out[:], stage[:])

    nc.compile()
    return nc


def _host_prep(x, labels, pe_mask=False):
    """Sort rows by label; window metadata + per-core input maps."""
    lab = np.asarray(labels).astype(np.int64)
    x = np.ascontiguousarray(np.asarray(x, dtype=np.float32))
    perm = np.argsort(lab, kind="stable")
    labs = lab[perm]
    xs = np.ascontiguousarray(x[perm])
    ws = np.searchsorted(labs, labs, "left").astype(np.int64)
    we = np.searchsorted(labs, labs, "right").astype(np.int64)
    labs_f = labs.astype(np.float32)

    if pe_mask:
        ohm = np.zeros((256, N), np.float32)
        ohm[labs, np.arange(N)] = 1.0

    in_maps = []
    los = np.zeros((NCORES, RT), dtype=np.int64)
    for m in range(NCORES):
        xpos = np.zeros((RT * POSW, D), np.float32)
        labp = np.zeros(RT * POSW, np.float32)
        labst = np.zeros((128, RT), np.float32)
        pos_cols = np.zeros(RT * POSW, np.int64)
        for rt in range(RT):
            base = m * STRIP + rt * 128
            rows = np.arange(base, base + 128)
            lo = min(max(base - (POSW - 128) // 2, 0), N - POSW)
            los[m, rt] = lo
            assert ws[rows].min() >= lo and we[rows].max() <= lo + POSW, (
                "positive window slice overflow; enlarge POSW"
            )
            xpos[rt * POSW:(rt + 1) * POSW] = xs[lo:lo + POSW]
            labp[rt * POSW:(rt + 1) * POSW] = labs_f[lo:lo + POSW]
            labst[:, rt] = labs_f[rows]
            pos_cols[rt * POSW:(rt + 1) * POSW] = np.arange(lo, lo + POSW)
        im = {
            "x_full": xs,
            "x_strip": np.ascontiguousarray(xs[m * STRIP:(m + 1) * STRIP]),
            "x_pos": xpos,
        }
        if pe_mask:
            strip_cols = np.arange(m * STRIP, (m + 1) * STRIP)
            ohs = np.ascontiguousarray(ohm[:, strip_cols]) * -2.0
            ohp = np.ascontiguousarray(ohm[:, pos_cols])
            im.update({
                "oh_full0": ohm[:128], "oh_full1": ohm[128:],
                "oh_strip0": ohs[:128], "oh_strip1": ohs[128:],
                "oh_pos0": ohp[:128], "oh_pos1": ohp[128:],
            })
        else:
            im.update({
                "lab_full": labs_f.reshape(1, N),
                "lab_pos": labp.reshape(1, RT * POSW),
                "lab_strip": labst,
            })
        in_maps.append(im)
    keep_sorted = ((we - ws) >= 2) & ((we - ws) <= N - 1)
    return in_maps, perm, los, keep_sorted


_CACHED_NC = None
_MASK_MODE = None


def kernel(l_embeds: np.ndarray, l_labels: np.ndarray):
    import os
    global _CACHED_NC, _MASK_MODE
    if _CACHED_NC is None:
        _MASK_MODE = os.environ.get("KERNEL_MASK_MODE", "dve")
        _CACHED_NC = build_program(mask_mode=_MASK_MODE)
    nc = _CACHED_NC

    lab_i = np.asarray(l_labels)
    in_maps, perm, los, keep_sorted = _host_prep(
        l_embeds, lab_i, pe_mask=_MASK_MODE == "pe")

    res = run_bass_kernel_spmd(nc, in_maps, list(range(NCORES))).results

    neg_sorted = np.empty(N, np.int64)
    pos_sorted = np.empty(N, np.int64)
    for m in range(NCORES):
        st = res[m]["stage_out"].astype(np.int64).reshape(128, RT, 8)
        for rt in range(RT):
            base = m * STRIP + rt * 128
            neg_sorted[base:base + 128] = st[:, rt, 0]
            pos_sorted[base:base + 128] = st[:, rt, 1]

    # sorted-space -> original-space indices
    idt = np.int64 if lab_i.dtype == np.int64 else np.int32
    anchor = np.arange(N, dtype=idt)
    neg = np.empty(N, np.int64)
    pos = np.empty(N, np.int64)
    keep = np.empty(N, bool)
    srows = np.arange(N)
    neg[perm[srows]] = perm[neg_sorted]
    pos[perm[srows]] = perm[pos_sorted]
    keep[perm[srows]] = keep_sorted
    return (anchor, pos.astype(idt), neg.astype(idt), keep)


# revision 25
# speedup vs baseline: 4.3921x; 2.2722x over previous
"""Hard-triplet miner for Trainium2, 8-core SPMD.

Per core: compute a [1024, 8192] strip of the Gram matrix G = x_norm @ x_norm.T
on the PE, then per 128-row tile build w = G - 2*[same_label] in one fused
DVE tensor_tensor_reduce pass (per-column-tile maxima as a byproduct).
Since sqrt/constant shifts are monotonic: hardest negative = argmax_j w,
hardest positive = argmin_j w.  Index extraction: one max_index pass whose
in_max carries BOTH the row max and the row min (max_index is a value
matcher).  keep = thresholds on the two extremes.
"""

import numpy as np

import concourse.bacc as bacc
import concourse.bass as bass
import concourse.mybir as mybir
import concourse.tile as tile
from concourse import masks
from concourse.bass_utils import run_bass_kernel_spmd

F32 = mybir.dt.float32
BF16 = mybir.dt.bfloat16
U32 = mybir.dt.uint32

N = 8192          # total rows
D = 128           # embed dim
NCORES = 8
STRIP = N // NCORES       # 1024 anchor rows per core
RT = STRIP // 128         # 8 row-tiles per core
CT_W = 1024               # column-tile width for psum/ttr
CT = N // CT_W            # 8 column tiles
NEG_INIT = -1.0e30
PAD_VAL = 3.0e38


def build_program(k_repeat: int = 1, use_for_i: bool = False, n: int = N,
                  strip: int = STRIP, debug_level: int = 0,
                  mask_f32: bool = False):
    """Build the SPMD program (identical on all cores).  n/strip shrinkable
    for simulator validation."""
    rt_n = strip // 128
    ct_n = n // CT_W if n >= CT_W else 1
    ct_w = min(CT_W, n)
    t_full = n // 128

    nc = bacc.Bacc("TRN2", target_bir_lowering=False, debug=False,
                   num_devices=NCORES)

    x_full = nc.dram_tensor("x_full", [n, D], F32, kind="ExternalInput")
    x_strip = nc.dram_tensor("x_strip", [strip, D], F32, kind="ExternalInput")
    lab_full = nc.dram_tensor("lab_full", [1, n], F32, kind="ExternalInput")
    lab_strip = nc.dram_tensor("lab_strip", [128, rt_n], F32,
                               kind="ExternalInput")
    neg_out = nc.dram_tensor("neg_out", [128, rt_n], U32, kind="ExternalOutput")
    pos_out = nc.dram_tensor("pos_out", [128, rt_n], U32, kind="ExternalOutput")
    keep_out = nc.dram_tensor("keep_out", [128, rt_n], F32,
                              kind="ExternalOutput")

    with tile.TileContext(nc) as tc:
        with (
            tc.tile_pool(name="persist", bufs=1) as persist,
            tc.tile_pool(name="rowp", bufs=3) as rowp,
            tc.tile_pool(name="maskp", bufs=2) as maskp,
            tc.tile_pool(name="nescp", bufs=1) as nescp,
            tc.tile_pool(name="wp", bufs=2) as wp,
            tc.tile_pool(name="smalls", bufs=4) as smalls,
            tc.tile_pool(name="psum_pro", bufs=2,
                         space=bass.MemorySpace.PSUM) as psum_pro,
            tc.tile_pool(name="psum_main", bufs=3,
                         space=bass.MemorySpace.PSUM) as psum_main,
        ):
            ident = persist.tile([128, 128], F32)
            masks.make_identity(nc, ident[:])

            xT = persist.tile([128, n], F32, tag="xT")
            xsT = persist.tile([128, strip], F32, tag="xsT")
            labrep = persist.tile([128, n], BF16, tag="labrep")
            labsT = persist.tile([128, rt_n], F32, tag="labsT")
            ones1 = persist.tile([1, 128], F32, tag="ones1")
            nc.gpsimd.memset(ones1[:], 1.0)
            lab1 = persist.tile([1, n], F32, tag="lab1")

            nc.sync.dma_start(lab1[:], lab_full[:])
            nc.sync.dma_start(labsT[:], lab_strip[:])

            bias2 = persist.tile([128, 1], F32, tag="bias2")
            nc.gpsimd.memset(bias2[:], 2.0)
            bias09 = persist.tile([128, 1], F32, tag="bias09")
            nc.gpsimd.memset(bias09[:], 0.9)
            biasm09 = persist.tile([128, 1], F32, tag="biasm09")
            nc.gpsimd.memset(biasm09[:], -0.9)
            bias0 = persist.tile([128, 1], F32, tag="bias0")
            nc.gpsimd.memset(bias0[:], 0.0)

            # --- normalize + transpose: build xT (all rows) and xsT (strip) ---
            def norm_transpose(dst, src_dram, tiles):
                for t in range(tiles):
                    row = rowp.tile([128, D], F32, tag="row")
                    nc.sync.dma_start(row[:], src_dram[t * 128:(t + 1) * 128, :])
                    sq = rowp.tile([128, D], F32, tag="sq")
                    ssq = smalls.tile([128, 1], F32, tag="ssq")
                    nc.scalar.activation(sq[:], row[:],
                                         mybir.ActivationFunctionType.Square,
                                         bias=bias0[:], accum_out=ssq[:])
                    nrm = smalls.tile([128, 1], F32, tag="nrm")
                    nc.scalar.activation(nrm[:], ssq[:],
                                         mybir.ActivationFunctionType.Sqrt,
                                         bias=bias0[:])
                    rin = smalls.tile([128, 1], F32, tag="rin")
                    nc.vector.reciprocal(rin[:], nrm[:])
                    xn = rowp.tile([128, D], F32, tag="xn")
                    nc.vector.tensor_scalar_mul(xn[:], row[:], rin[:])
                    pt = psum_pro.tile([128, 512], F32, tag="ppro")
                    nc.tensor.transpose(pt[:, 0:128], xn[:], ident[:])
                    nc.scalar.activation(dst[:, t * 128:(t + 1) * 128],
                                         pt[:, 0:128],
                                         mybir.ActivationFunctionType.Copy)

            norm_transpose(xT, x_full, t_full)
            norm_transpose(xsT, x_strip, rt_n)

            # --- replicate labels across partitions (matmul broadcast) ---
            for c in range(n // 512):
                pl = psum_pro.tile([128, 512], F32, tag="ppro")
                nc.tensor.matmul(pl[:], ones1[:], lab1[:, c * 512:(c + 1) * 512])
                nc.scalar.activation(labrep[:, c * 512:(c + 1) * 512], pl[:],
                                     mybir.ActivationFunctionType.Copy)

            labsTm2 = persist.tile([128, rt_n], F32, tag="labsTm2")
            nc.vector.tensor_scalar_mul(labsTm2[:], labsT[:], -2.0)

            neg_stage = persist.tile([128, rt_n], U32, tag="neg_stage")
            pos_stage = persist.tile([128, rt_n], U32, tag="pos_stage")
            keep_stage = persist.tile([128, rt_n], F32, tag="keep_stage")

            def main_body():
                for rt in range(rt_n):
                    if debug_level >= 3:
                        nc.vector.memset(neg_stage[:, rt:rt + 1], 0)
                        nc.vector.memset(pos_stage[:, rt:rt + 1], 0)
                        nc.vector.memset(keep_stage[:, rt:rt + 1], 0)
                        continue
                    # nesc = |2*lab_j - 2*lab_i|; eqsc = relu(2 - nesc)
                    # => 2.0 where labels equal, 0 where different
                    nesc = nescp.tile([128, n], BF16, tag="nesc")
                    nc.scalar.activation(nesc[:], labrep[:],
                                         mybir.ActivationFunctionType.Abs,
                                         scale=2.0,
                                         bias=labsTm2[:, rt:rt + 1])
                    eqsc = maskp.tile([128, n], F32 if mask_f32 else BF16,
                                      tag="eqsc")
                    nc.scalar.activation(eqsc[:], nesc[:],
                                         mybir.ActivationFunctionType.Relu,
                                         scale=-1.0, bias=bias2[:])
                    w = wp.tile([128, n], F32, tag="w")
                    slots = smalls.tile([128, 8], F32, tag="slots")
                    if debug_level == 2:
                        nc.vector.memset(slots[:], 0)
                        nc.vector.tensor_copy(neg_stage[:, rt:rt + 1],
                                              slots[:, 0:1])
                        nc.vector.tensor_copy(pos_stage[:, rt:rt + 1],
                                              slots[:, 1:2])
                        nc.vector.tensor_copy(keep_stage[:, rt:rt + 1],
                                              slots[:, 2:3])
                        continue
                    for ct in range(ct_n):
                        ps = psum_main.tile([128, ct_w], F32, tag="ps")
                        for h in range(ct_w // 512):
                            lo = ct * ct_w + h * 512
                            nc.tensor.matmul(
                                ps[:, h * 512:(h + 1) * 512],
                                xsT[:, rt * 128:(rt + 1) * 128],
                                xT[:, lo:lo + 512])
                        if debug_level == 4:
                            nc.scalar.activation(
                                w[:, ct * ct_w:(ct + 1) * ct_w], ps[:],
                                mybir.ActivationFunctionType.Copy)
                            continue
                        nc.vector.tensor_tensor(
                            w[:, ct * ct_w:(ct + 1) * ct_w], ps[:],
                            eqsc[:, ct * ct_w:(ct + 1) * ct_w],
                            mybir.AluOpType.subtract)
                    if debug_level >= 1:
                        nc.vector.memset(slots[:], 0)
                        nc.vector.tensor_copy(neg_stage[:, rt:rt + 1],
                                              slots[:, 0:1])
                        nc.vector.tensor_copy(pos_stage[:, rt:rt + 1],
                                              slots[:, 1:2])
                        nc.vector.tensor_copy(keep_stage[:, rt:rt + 1],
                                              slots[:, 2:3])
                        continue
                    # ---- extraction ----
                    top8 = smalls.tile([128, 1], F32, tag="top8")
                    nc.vector.tensor_reduce(top8[:], w[:], mybir.AxisListType.X,
                                            mybir.AluOpType.max)
                    gmin = smalls.tile([128, 1], F32, tag="gmin")
                    nc.vector.tensor_reduce(gmin[:], w[:], mybir.AxisListType.X,
                                            mybir.AluOpType.min)
                    inmax = smalls.tile([128, 8], F32, tag="inmax")
                    nc.vector.memset(inmax[:], PAD_VAL)
                    nc.vector.tensor_copy(inmax[:, 0:1], top8[:, 0:1])
                    nc.vector.tensor_copy(inmax[:, 1:2], gmin[:])
                    idx8 = smalls.tile([128, 8], U32, tag="idx8")
                    nc.vector.max_index(idx8[:], inmax[:], w[:])
                    nc.vector.tensor_copy(neg_stage[:, rt:rt + 1], idx8[:, 0:1])
                    nc.vector.tensor_copy(pos_stage[:, rt:rt + 1], idx8[:, 1:2])
                    # keep_neg = (gmax > -0.9), keep_pos = (gmin < -0.9)
                    kn = smalls.tile([128, 1], F32, tag="kn")
                    nc.scalar.activation(kn[:], top8[:],
                                         mybir.ActivationFunctionType.Sign,
                                         scale=1.0, bias=bias09[:])
                    nc.scalar.activation(kn[:], kn[:],
                                         mybir.ActivationFunctionType.Relu,
                                         bias=bias0[:])
                    kp = smalls.tile([128, 1], F32, tag="kp")
                    nc.scalar.activation(kp[:], gmin[:],
                                         mybir.ActivationFunctionType.Sign,
                                         scale=-1.0, bias=biasm09[:])
                    nc.scalar.activation(kp[:], kp[:],
                                         mybir.ActivationFunctionType.Relu,
                                         bias=bias0[:])
                    nc.vector.tensor_tensor(keep_stage[:, rt:rt + 1], kn[:],
                                            kp[:], mybir.AluOpType.mult)

            if use_for_i:
                with tc.For_i(0, k_repeat, 1):
                    main_body()
            else:
                for _ in range(k_repeat):
                    main_body()

            nc.sync.dma_start(neg_out[:], neg_stage[:])
            nc.sync.dma_start(pos_out[:], pos_stage[:])
            nc.sync.dma_start(keep_out[:], keep_stage[:])

    nc.compile()
    return nc


_CACHED_NC = None


def kernel(l_embeds: np.ndarray, l_labels: np.ndarray):
    global _CACHED_NC
    if _CACHED_NC is None:
        _CACHED_NC = build_program()
    nc = _CACHED_NC

    x = np.ascontiguousarray(np.asarray(l_embeds, dtype=np.float32))
    lab_i = np.asarray(l_labels)
    lab = lab_i.astype(np.float32)

    in_maps = []
    for m in range(NCORES):
        sl = slice(m * STRIP, (m + 1) * STRIP)
        in_maps.append({
            "x_full": x,
            "x_strip": np.ascontiguousarray(x[sl]),
            "lab_full": lab.reshape(1, N),
            # lab_strip[p, r] = lab[m*STRIP + r*128 + p]
            "lab_strip": np.ascontiguousarray(
                lab[sl].reshape(RT, 128).T),
        })

    res = run_bass_kernel_spmd(nc, in_maps, list(range(NCORES))).results

    neg = np.empty(N, np.int64)
    pos = np.empty(N, np.int64)
    keep = np.empty(N, np.float32)
    for m in range(NCORES):
        sl = slice(m * STRIP, (m + 1) * STRIP)
        # stage[p, r] -> row r*128+p  =>  transpose to [rt, 128] then flatten
        neg[sl] = res[m]["neg_out"].T.reshape(-1)
        pos[sl] = res[m]["pos_out"].T.reshape(-1)
        keep[sl] = res[m]["keep_out"].T.reshape(-1)

    idt = np.int32 if lab_i.dtype != np.int64 else np.int64
    anchor = np.arange(N, dtype=idt)
    return (anchor, pos.astype(idt), neg.astype(idt), keep > 0.5)
